# revision 1
# baseline (speedup 1.0000x reference)
"""CoAttention Trainium2 kernel.

Problem: B=16, PLEN=1024, QLEN=256, D=256 fp32.
  score[b,p,q] = passage.w_p + question.w_q + (passage*w_pq).question + b
  masked-softmax both ways, three attention matmuls.

Strategy: data-parallel over batch across 8 NeuronCores (2 batches/core).
Per batch on one core everything is local (no collectives):

  - PE-transpose P -> PT [d,p] and Q -> QT [d,q]; QwT = QT * w_pq (the
    elementwise w_pq weight is folded into the *question* side).
  - S0[p,q] = PT.T @ QwT  (fp32r matmuls, K=d=256); an extra rhs column
    (w_p) makes the same psum deliver sp = P@w_p.
  - Softmax factorization: with g[q] = exp(sq - 1e7*qm + b) and
    h[p] = exp(sp) * (1-pm), the masked-softmax weights are
      p2q[p,q] = E[p,q]*g[q] / (E@g)[p],   E  = exp(S0)
      q2p[q,p] = ET[q,p]*h[p] / (ET@h)[q], ET = exp(QwT.T @ PT)
    (row-constant factors cancel inside softmax; masks enter as exact
    zeros in g/h since exp(-1e7) == 0 on ACT).
  - p2q_att = diag(kp/dp) . E @ [Q*g | g]      (denominator via extra col)
    q2p_att = diag(g/dq)  . ET @ [P*h | h]
    coatt   = diag(kp/dp) . E @ q2p_att
    Normalization scales are per-partition and ride the PSUM->SBUF copies.

Scheduling: P is DMA'd in two halves; per half the pipeline is
PT-transpose -> S0 -> E=exp -> ST0-chunk -> ET=exp -> p2q -> stream out,
so PE starts on Q transposes ~2us in and outputs drain mid-kernel.

The container's walrus accepts only ONE sync-wait per non-matmul
instruction (and none on fp32r matmuls with odd moving dims); a BIR
post-pass splits waits into single-wait EventSemaphore carriers and all
matmul moving dims are padded to even sizes.
"""

import numpy as np
import orjson

import concourse.bass as bass
import concourse.mybir as mybir
import concourse.tile as tile
from concourse.bass_utils import run_bass_kernel_spmd
from concourse.masks import make_identity

F32 = mybir.dt.float32
F32R = mybir.dt.float32r
I32 = mybir.dt.int32
AF = mybir.ActivationFunctionType

N_CORES = 8
B, PLEN, QLEN, D = 16, 1024, 256, 256
NB = B // N_CORES  # batches per core
PT_T = PLEN // 128  # 8 p-tiles
QT_T = QLEN // 128  # 2 q-tiles
DT_T = D // 128  # 2 d-tiles
MASK = -10000000.0
EPS = 1e-30

# ---------------------------------------------------------------------------
# walrus single-wait workaround


def _split_waits_in_bir(bir: dict) -> None:
    for f in bir.get("functions", []):
        for blk in f.get("blocks", []):
            out = []
            for i in blk.get("instructions", []):
                si = i.get("sync_info")
                ow = (si or {}).get("on_wait") or []
                limit = 0 if i.get("opcode") == "Matmult" else 1
                if len(ow) > limit:
                    for k, w in enumerate(ow[limit:]):
                        out.append(
                            {
                                "debug": i.get("debug"),
                                "engine": i["engine"],
                                "ins": [],
                                "outs": [],
                                "name": f"{i['name']}__w{k}",
                                "opcode": "EventSemaphore",
                                "sync_info": {"on_update": [], "on_wait": [w]},
                            }
                        )
                    si["on_wait"] = ow[:limit]
                out.append(i)
            blk["instructions"] = out


_patched = False


def _install_bir_wait_split():
    global _patched
    if _patched:
        return
    _patched = True
    import concourse.bass2jax as b2j
    import concourse.bass_utils as bu

    orig = bu.compile_bir_kernel

    def patched(bir_json, tmpdir, neff_name="file.neff"):
        bir = orjson.loads(bir_json)
        _split_waits_in_bir(bir)
        return orig(orjson.dumps(bir), tmpdir, neff_name)

    bu.compile_bir_kernel = patched
    b2j.compile_bir_kernel = patched


# ---------------------------------------------------------------------------


def build_nc(bufs_cfg=None) -> bass.Bass:
    cfg = {"tp": 2, "s0": 1, "st": 1, "at": 2, "co": 2, "big": 2, "small": 2}
    if bufs_cfg:
        cfg.update(bufs_cfg)
    nc = bass.Bass()
    passage = nc.declare_dram_parameter("passage", [NB, PLEN, D], F32, isOutput=False)
    question = nc.declare_dram_parameter("question", [NB, QLEN, D], F32, isOutput=False)
    pmask = nc.declare_dram_parameter("passage_mask", [NB, PLEN], I32, isOutput=False)
    qmask = nc.declare_dram_parameter("question_mask", [NB, QLEN], I32, isOutput=False)
    w_all = nc.declare_dram_parameter("W", [3 * D], F32, isOutput=False)
    b_in = nc.declare_dram_parameter("b", [1], F32, isOutput=False)
    out_p2q = nc.declare_dram_parameter("p2q", [NB, PLEN, D], F32, isOutput=True)
    out_co = nc.declare_dram_parameter("coatt", [NB, PLEN, D], F32, isOutput=True)

    with tile.TileContext(nc) as tc:
        with (
            tc.tile_pool(name="const", bufs=1) as const_pool,
            tc.tile_pool(name="big", bufs=cfg["big"]) as big,
            tc.tile_pool(name="small", bufs=cfg["small"]) as small,
            tc.tile_pool(name="tp_ps", bufs=cfg["tp"], space="PSUM") as tp_ps,
            tc.tile_pool(name="s0_ps", bufs=cfg["s0"], space="PSUM") as s0_ps,
            tc.tile_pool(name="st_ps", bufs=cfg["st"], space="PSUM") as st_ps,
            tc.tile_pool(name="at_ps", bufs=cfg["at"], space="PSUM") as at_ps,
            tc.tile_pool(name="co_ps", bufs=cfg["co"], space="PSUM") as co_ps,
        ):
            ident = const_pool.tile([128, 128], F32, name="ident")
            make_identity(nc, ident[:])
            ident_r_t = const_pool.tile([128, 128], F32R, name="ident_r_t")
            nc.vector.tensor_copy(ident_r_t[:], ident[:])
            ident_r = ident_r_t[:]

            # weight columns: [d_in_tile, k]  cols: wp0 wp1 wq0 wq1 wpq0 wpq1
            w6 = const_pool.tile([128, 6], F32, name="w6")
            nc.gpsimd.dma_start(w6[:], w_all[:].rearrange("(k d) -> d k", d=128))
            w_p = w6[:, 0:DT_T]
            w_pq = w6[:, 2 * DT_T : 3 * DT_T]
            w_q_r = const_pool.tile([128, DT_T + 1], F32R, name="w_q_r")
            nc.gpsimd.memset(w_q_r[:].bitcast(F32), 0.0)
            nc.vector.tensor_copy(w_q_r[:, 0:DT_T], w6[:, DT_T : 2 * DT_T])
            b_sb = const_pool.tile([128, 1], F32, name="b_sb")

            # ---- batched loads: masks for all batches --------------------
            pm_all = const_pool.tile([128, NB, PT_T], I32, name="pm_all")
            nc.gpsimd.dma_start(
                pm_all[:], pmask[:].rearrange("n (t p) -> p n t", p=128)
            )
            qm_all = const_pool.tile([128, NB, QT_T], I32, name="qm_all")
            nc.gpsimd.dma_start(
                qm_all[:], qmask[:].rearrange("n (t q) -> q n t", q=128)
            )
            nc.gpsimd.dma_start(b_sb[:], b_in[0:1].partition_broadcast(128))

            def emit_batch(bi):
                p2q_dst = out_p2q[bi].rearrange("(t p) d -> p t d", p=128)
                co_dst = out_co[bi].rearrange("(t p) d -> p t d", p=128)
                # ---- loads (SP queue order == emission order) -----------
                q_sb = small.tile([128, QT_T, D], F32R, name="q_sb", tag="q_sb")
                q_src = question[bi].rearrange("(t q) d -> q t d", q=128).bitcast(F32R)
                for t4 in range(QT_T):
                    nc.sync.dma_start(q_sb[:, t4 : t4 + 1, :], q_src[:, t4 : t4 + 1, :])
                p_sb = big.tile([128, PT_T, D], F32R, name="p_sb", tag="p_sb")
                p_src = passage[bi].rearrange("(t p) d -> p t d", p=128).bitcast(F32R)
                for half in range(2):
                    nc.sync.dma_start(
                        p_sb[:, half * 4 : (half + 1) * 4, :],
                        p_src[:, half * 4 : (half + 1) * 4, :],
                    )

                # ---- mask vectors ---------------------------------------
                pm_f = small.tile([128, PT_T], F32, name="pm_f", tag="pm_f")
                nc.vector.tensor_copy(pm_f[:], pm_all[:, bi])
                kp = small.tile([128, PT_T], F32, name="kp", tag="kp")
                nc.vector.tensor_scalar(
                    kp[:], pm_f[:], -1.0, 1.0, mybir.AluOpType.mult, mybir.AluOpType.add
                )
                qm_f = small.tile([128, QT_T], F32, name="qm_f", tag="qm_f")
                nc.vector.tensor_copy(qm_f[:], qm_all[:, bi])
                qmb = small.tile([128, QT_T], F32, name="qmb", tag="qmb")
                nc.vector.tensor_scalar(
                    qmb[:],
                    qm_f[:],
                    MASK,
                    b_sb[:, 0:1],
                    mybir.AluOpType.mult,
                    mybir.AluOpType.add,
                )

                # ---- QT transposes (only need q_sb) ---------------------
                qt_r = small.tile([128, DT_T, QLEN], F32R, name="qt_r", tag="qt_r")
                qwt = small.tile([128, DT_T, QLEN + 2], F32R, name="qwt", tag="qwt")
                tqs = [
                    tp_ps.tile([128, 256], F32R, name=f"tq{j}", tag="tp")
                    for j in range(DT_T)
                ]
                for t4 in range(QT_T):
                    for j in range(DT_T):
                        nc.tensor.transpose(
                            tqs[j][:, t4 * 128 : (t4 + 1) * 128],
                            q_sb[:, t4, j * 128 : (j + 1) * 128],
                            ident_r,
                        )
                for j in range(DT_T):
                    tq = tqs[j]
                    nc.vector.tensor_copy(qt_r[:, j, :], tq[:])
                    # QwT = QT * w_pq (per-partition d scale), fp32r rounded
                    nc.vector.tensor_scalar_mul(
                        qwt[:, j, 0:QLEN], tq[:], w_pq[:, j : j + 1]
                    )
                    nc.vector.tensor_copy(qwt[:, j, QLEN : QLEN + 1], w_p[:, j : j + 1])
                    nc.vector.tensor_copy(
                        qwt[:, j, QLEN + 1 : QLEN + 2], w_p[:, j : j + 1]
                    )

                # ---- sq = Q @ w_q, g = exp(sq - 1e7*qm + b) -------------
                qgg = small.tile([128, QT_T, QLEN + 2], F32R, name="qgg", tag="qgg")
                sq = tp_ps.tile([128, QT_T, 2], F32, name="sq", tag="tp")
                for tq_i in range(QT_T):
                    for j in range(DT_T):
                        nc.tensor.matmul(
                            sq[:, tq_i, 0:2],
                            qt_r[:, j, tq_i * 128 : (tq_i + 1) * 128],
                            w_q_r[:, j : j + 2],
                            start=(j == 0),
                            stop=(j == DT_T - 1),
                        )
                for tq_i in range(QT_T):
                    nc.scalar.activation(
                        qgg[:, tq_i, QLEN : QLEN + 1],
                        sq[:, tq_i, 0:1],
                        AF.Exp,
                        bias=qmb[:, tq_i : tq_i + 1],
                    )
                    nc.gpsimd.tensor_copy(
                        qgg[:, tq_i, QLEN + 1 : QLEN + 2], qgg[:, tq_i, QLEN : QLEN + 1]
                    )
                    # Qg = Q * g (per-partition q scale)
                    nc.gpsimd.tensor_scalar_mul(
                        qgg[:, tq_i, 0:QLEN],
                        q_sb[:, tq_i, :].bitcast(F32),
                        qgg[:, tq_i, QLEN : QLEN + 1].bitcast(F32),
                    )

                yield  # head done (loads, masks, QT, sq, g, Qgg)

                # ---- per p-half: PT, S0, E, h/Ph, ET, p2q ---------------
                pt_r = big.tile([128, DT_T, PLEN], F32R, name="pt_r", tag="pt_r")
                e_sb = big.tile([128, PT_T, QLEN + 2], F32R, name="e_sb", tag="e_sb")
                et_sb = big.tile([128, QT_T, PLEN], F32R, name="et_sb", tag="et_sb")
                phh = big.tile([128, PT_T, D + 2], F32R, name="phh", tag="phh")
                p2q_sb = big.tile([128, PT_T, D], F32, name="p2q_sb", tag="p2q_sb")
                co_sb = big.tile([128, PT_T, D], F32, name="co_sb", tag="co_sb")
                rp = small.tile([128, PT_T], F32, name="rp", tag="rp")

                def emit_coatt(t, co_sb=co_sb, co_dst=co_dst, rp=rp):
                    co = co_ps.tile([128, D], F32, name="co", tag="co")
                    for tq_i in range(QT_T):
                        nc.tensor.matmul(
                            co[:],
                            et_sb[:, tq_i, t * 128 : (t + 1) * 128],
                            q2p[:, tq_i, :],
                            start=(tq_i == 0),
                            stop=(tq_i == QT_T - 1),
                        )
                    if t % 2 == 0:
                        nc.scalar.activation(
                            co_sb[:, t, :], co[:], AF.Copy, scale=rp[:, t : t + 1]
                        )
                    else:
                        nc.vector.tensor_scalar_mul(
                            co_sb[:, t, :], co[:], rp[:, t : t + 1]
                        )
                    if t % 2 == 1:
                        nc.gpsimd.dma_start(
                            co_dst[:, t - 1 : t + 1, :], co_sb[:, t - 1 : t + 1, :]
                        )
                for grp in range(2):
                    t_lo = grp * 4
                    # PT transposes for this half
                    for j in range(DT_T):
                        tp = tp_ps.tile([128, 512], F32R, name="tp", tag="tp")
                        for t4 in range(4):
                            t = t_lo + t4
                            nc.tensor.transpose(
                                tp[:, t4 * 128 : (t4 + 1) * 128],
                                p_sb[:, t, j * 128 : (j + 1) * 128],
                                ident_r,
                            )
                        if j == 0:
                            nc.vector.tensor_copy(
                                pt_r[:, j, grp * 512 : (grp + 1) * 512], tp[:]
                            )
                        else:
                            nc.scalar.copy(
                                pt_r[:, j, grp * 512 : (grp + 1) * 512], tp[:]
                            )
                    # scores S0 (+ sp column), E = exp(S0), h, Ph
                    for t in range(t_lo, t_lo + 4):
                        s0 = s0_ps.tile([128, QLEN + 2], F32, name="s0", tag="s0")
                        for j in range(DT_T):
                            nc.tensor.matmul(
                                s0[:],
                                pt_r[:, j, t * 128 : (t + 1) * 128],
                                qwt[:, j, :],
                                start=(j == 0),
                                stop=(j == DT_T - 1),
                            )
                        nc.scalar.activation(e_sb[:, t, :], s0[:], AF.Exp)
                        # h = exp(sp) * kp  (mask as multiplicative zero)
                        nc.gpsimd.tensor_mul(
                            phh[:, t, D : D + 1],
                            e_sb[:, t, QLEN : QLEN + 1].bitcast(F32),
                            kp[:, t : t + 1],
                        )
                        nc.gpsimd.tensor_copy(
                            phh[:, t, D + 1 : D + 2], phh[:, t, D : D + 1]
                        )
                        nc.gpsimd.tensor_scalar_mul(
                            phh[:, t, 0:D],
                            p_sb[:, t, :].bitcast(F32),
                            phh[:, t, D : D + 1].bitcast(F32),
                        )
                    if grp == 1:
                        # ---- q2p attention [q,d] (needs all of E/Ph) ----
                        q2p = small.tile([128, QT_T, D], F32R, name="q2p", tag="q2p")
                        s_vec = small.tile([128, QT_T], F32, name="s_vec", tag="s_vec")
                        for tq_i in range(QT_T):
                            aq = at_ps.tile([128, D + 2], F32, name="aq", tag="at")
                            for t in range(PT_T):
                                nc.tensor.matmul(
                                    aq[:],
                                    e_sb[:, t, tq_i * 128 : (tq_i + 1) * 128],
                                    phh[:, t, :],
                                    start=(t == 0),
                                    stop=(t == PT_T - 1),
                                )
                            u1 = small.tile([128, 1], F32, name="u1", tag="u1")
                            nc.vector.tensor_scalar_add(
                                u1[:], aq[:, D : D + 1], EPS
                            )
                            u2 = small.tile([128, 1], F32, name="u2", tag="u2")
                            nc.vector.reciprocal(u2[:], u1[:])
                            nc.vector.tensor_mul(
                                s_vec[:, tq_i : tq_i + 1],
                                u2[:],
                                qgg[:, tq_i, QLEN : QLEN + 1],
                            )
                            nc.vector.tensor_scalar_mul(
                                q2p[:, tq_i, :],
                                aq[:, 0:D],
                                s_vec[:, tq_i : tq_i + 1],
                            )
                    # ST0 chunk for this half: ET[:, :, grp cols] = exp(ST0)
                    for tq_i in range(QT_T):
                        st = st_ps.tile([128, 512], F32, name="st", tag="st")
                        for j in range(DT_T):
                            nc.tensor.matmul(
                                st[:],
                                qwt[:, j, tq_i * 128 : (tq_i + 1) * 128],
                                pt_r[:, j, grp * 512 : (grp + 1) * 512],
                                start=(j == 0),
                                stop=(j == DT_T - 1),
                            )
                        nc.scalar.activation(
                            et_sb[:, tq_i, grp * 512 : (grp + 1) * 512], st[:], AF.Exp
                        )
                    # p2q attention for this half's p-tiles (+ early coatt
                    # interleaved in the second half)
                    for t in range(t_lo, t_lo + 4):
                        ap_ = at_ps.tile([128, QLEN + 2], F32, name="ap_", tag="at")
                        for tq_i in range(QT_T):
                            nc.tensor.matmul(
                                ap_[:],
                                et_sb[:, tq_i, t * 128 : (t + 1) * 128],
                                qgg[:, tq_i, :],
                                start=(tq_i == 0),
                                stop=(tq_i == QT_T - 1),
                            )
                        v1 = small.tile([128, 1], F32, name="v1", tag="v1")
                        nc.vector.tensor_scalar_add(v1[:], ap_[:, QLEN : QLEN + 1], EPS)
                        v2 = small.tile([128, 1], F32, name="v2", tag="v2")
                        nc.vector.reciprocal(v2[:], v1[:])
                        nc.vector.tensor_mul(rp[:, t : t + 1], v2[:], kp[:, t : t + 1])
                        nc.vector.tensor_scalar_mul(
                            p2q_sb[:, t, :], ap_[:, 0:QLEN], rp[:, t : t + 1]
                        )
                        if grp == 1:
                            if t % 2 == 1:
                                nc.sync.dma_start(
                                    p2q_dst[:, t - 1 : t + 1, :],
                                    p2q_sb[:, t - 1 : t + 1, :],
                                )
                            emit_coatt(t - 4)
                    if grp == 0:
                        nc.sync.dma_start(
                            p2q_dst[:, t_lo : t_lo + 4, :],
                            p2q_sb[:, t_lo : t_lo + 4, :],
                        )
                        yield  # first p-half done

                # ---- coattention second half ----------------------------
                for t in range(4, PT_T):
                    emit_coatt(t)
                yield  # batch complete

            # Sequential per-batch emission measured fastest; Tile's
            # scheduler handles cross-batch overlap via the bufs=2 pools.
            for bi in range(NB):
                for _ in emit_batch(bi):
                    pass

    return nc


_nc_cache = None


def kernel(passage, question, passage_mask, question_mask, W, b):
    global _nc_cache
    _install_bir_wait_split()
    if _nc_cache is None:
        _nc_cache = build_nc()
    nc = _nc_cache

    passage = np.ascontiguousarray(passage, dtype=np.float32)
    question = np.ascontiguousarray(question, dtype=np.float32)
    passage_mask = np.ascontiguousarray(passage_mask, dtype=np.int32)
    question_mask = np.ascontiguousarray(question_mask, dtype=np.int32)
    W = np.ascontiguousarray(W, dtype=np.float32)
    b = np.ascontiguousarray(b, dtype=np.float32)

    in_maps = []
    for c in range(N_CORES):
        s = slice(c * NB, (c + 1) * NB)
        in_maps.append(
            {
                "passage": passage[s],
                "question": question[s],
                "passage_mask": passage_mask[s],
                "question_mask": question_mask[s],
                "W": W,
                "b": b,
            }
        )
    res = run_bass_kernel_spmd(nc, in_maps, list(range(N_CORES)))
    p2q = np.concatenate([r["p2q"] for r in res.results], axis=0)
    coatt = np.concatenate([r["coatt"] for r in res.results], axis=0)
    return p2q, coatt

